# revision 1
# baseline (speedup 1.0000x reference)
"""Trainium2 Bass kernel for nn_Attention_Rel_Scl (B=8,S=1024,E=1024,H=16).

Data-parallel over batch: one batch element per NeuronCore (8 cores).

Key ideas:
  - All matmuls fp16 (1 cyc/row on the PE; ~4e-4 end-to-end rel err).
  - Attention computed transposed (attnT[j, i]) so the softmax denominator
    is a PE column-sum (matmul with a ones stationary) and P@V needs no
    transposes of the 16M-element attention tensor.
  - No max-subtraction in softmax (logits ~ N(0, 0.1) => exp safe);
    mathematically identical to jax.nn.softmax.
  - The (h,S,S) relative bias is never materialized. Verified identity:
        bias[h,i,j] = flat[(15360 - 1024*h) + 1024*(i%16) - 16*(i//16) + j]
    with flat = rel_table.reshape(-1) (clip in the reference never fires).
    Post-softmax bias distributes: out = (P@v)/s + bias@v, and bias@v is a
    plain PE matmul whose moving operand is a strided view into a diagonal
    SBUF buffer T2[p, w] = flat[p + w] (no bias DMA traffic at all).
  - Query rows are processed in permuted order sigma(f) = 16*(63 - f%64) +
    f//64, which makes the T2 view affine; the host un-permutes output rows.
  - LayerNorm fused at the end: PE transposes -> bn_stats/bn_aggr -> apply.
"""

import sys

if "/opt/trn_rl_repo" not in sys.path:
    sys.path.insert(0, "/opt/trn_rl_repo")

import numpy as np

B, S, E, H = 8, 1024, 1024, 16
D = E // H          # 64 head dim
P = 128             # partitions
G = H // 2          # 8 head pairs
NBLK = S // P       # 8 key blocks
KBLK = E // P       # 8 contraction blocks
EPS = 1e-3
SCALE = float(E) ** -0.5
FLAT = (2 * S - 1) * H   # 32752
T2W = 32625              # max free offset 32624 (+p<=127 -> 32751 = FLAT-1)

# processing position f -> true query row (within each batch row space)
_f = np.arange(S)
SIGMA = 16 * (63 - _f % 64) + _f // 64
# stored partition p -> row offset within its 128-row output block
_p = np.arange(P)
ROWMAP = 16 * (7 - _p % 8) + _p // 8

_BUILT = {}
ABL = set()  # ablation flags for perf attribution (dev only)
LVL = 3      # 0: proj only, 1: +qk/exp, 2: +colsum/pvp/combine, 3: full


def _build(trivial_ln: bool):
    import concourse.bass as bass
    import concourse.tile as tile
    from concourse import bacc, mybir
    from concourse.masks import make_identity
    from contextlib import ExitStack

    fp16 = mybir.dt.float16
    fp32 = mybir.dt.float32
    Exp = mybir.ActivationFunctionType.Exp
    Sqrt = mybir.ActivationFunctionType.Sqrt
    mult = mybir.AluOpType.mult
    add = mybir.AluOpType.add
    sub = mybir.AluOpType.subtract

    nc = bacc.Bacc("TRN2", target_bir_lowering=False, debug=False,
                   num_devices=8)

    xt16 = nc.dram_tensor("xt16", [E, S], fp16, kind="ExternalInput").ap()
    xtp16 = nc.dram_tensor("xtp16", [E, S], fp16, kind="ExternalInput").ap()
    wq16 = nc.dram_tensor("wq16", [E, E], fp16, kind="ExternalInput").ap()
    wk16 = nc.dram_tensor("wk16", [E, E], fp16, kind="ExternalInput").ap()
    wv16 = nc.dram_tensor("wv16", [E, E], fp16, kind="ExternalInput").ap()
    flat16 = nc.dram_tensor("flat16", [FLAT], fp16, kind="ExternalInput").ap()
    if not trivial_ln:
        gam = nc.dram_tensor("gamma", [1, E], fp32, kind="ExternalInput").ap()
        bet = nc.dram_tensor("beta", [1, E], fp32, kind="ExternalInput").ap()
    out = nc.dram_tensor("out", [S, E], fp32, kind="ExternalOutput").ap()

    with tile.TileContext(nc) as tc, ExitStack() as ctx:
        persist = ctx.enter_context(tc.tile_pool(name="persist", bufs=1))
        T2 = persist.tile([P, T2W], fp16, name="T2")        # 63.7 KB/p
        QT = persist.tile([P, G, S], fp16, name="QT")       # 16 KB/p
        KT = persist.tile([P, G, S], fp16, name="KT")       # 16 KB/p
        V = persist.tile([P, NBLK, E], fp16, name="V")      # 16 KB/p
        ones16 = persist.tile([P, D], fp16, name="ones16")
        ident = persist.tile([P, P], fp32, name="ident")
        epsT = persist.tile([P, 1], fp32, name="epsT")

        nc.vector.memset(ones16, 1.0)
        nc.vector.memset(epsT, EPS)
        make_identity(nc, ident)

        # T2[p, w] = flat[p + w]  (one overlapping-read DMA, ~8.3 MB)
        nc.sync.dma_start(
            out=T2,
            in_=bass.AP(tensor=flat16.tensor, offset=0,
                        ap=[[1, P], [1, T2W]]),
        )

        if not trivial_ln:
            gamT = persist.tile([P, E], fp32, name="gamT")
            betT = persist.tile([P, E], fp32, name="betT")
            nc.sync.dma_start(
                out=gamT,
                in_=bass.AP(tensor=gam.tensor, offset=0, ap=[[0, P], [1, E]]),
            )
            nc.sync.dma_start(
                out=betT,
                in_=bass.AP(tensor=bet.tensor, offset=0, ap=[[0, P], [1, E]]),
            )

        psQ = ctx.enter_context(
            tc.tile_pool(name="psQ", bufs=2, space="PSUM"))
        psAcc = ctx.enter_context(
            tc.tile_pool(name="psAcc", bufs=4, space="PSUM"))

        def ptile():
            return psAcc.tile([P, 512], fp32, tag="acc512", name="acc512")

        def qtile():
            return psQ.tile([P, E], fp32, tag="qk1024", name="qk1024")

        # ============ Stage 1: xT / xTp, Q^T, K^T, V projections ==========
        with tc.tile_pool(name="s1fix", bufs=1) as s1fix, \
             tc.tile_pool(name="wpool", bufs=6) as wpool:
            xT = s1fix.tile([P, KBLK, S], fp16, name="xT")
            xTp = s1fix.tile([P, KBLK, S], fp16, name="xTp")
            wv_sb = s1fix.tile([P, KBLK, E], fp16, name="wv_sb")
            nc.sync.dma_start(
                out=xT, in_=xt16.rearrange("(kb kp) s -> kp kb s", kp=P))
            nc.sync.dma_start(
                out=xTp, in_=xtp16.rearrange("(kb kp) s -> kp kb s", kp=P))
            nc.sync.dma_start(
                out=wv_sb,
                in_=wv16.rearrange("(kb kp) e -> kp kb e", kp=P),
            )

            # Interleaved: per pair g emit QT(g), KT(g), then two V
            # j-blocks, so stage-2 QK for pair g starts ASAP while V
            # completes by the end of stage 1.
            for g in range(G):
                for wdram, dst, rhs_src in (
                    (wq16, QT, xTp), (wk16, KT, xT),
                ):
                    wt = wpool.tile([P, KBLK, P], fp16, tag="wtile",
                                    name="wt")
                    nc.sync.dma_start(
                        out=wt,
                        in_=wdram.rearrange("(kb kp) e -> kp kb e", kp=P)[
                            :, :, g * P:(g + 1) * P],
                    )
                    for ic in range(2):
                        pt = ptile()
                        for kb in range(KBLK):
                            nc.tensor.matmul(
                                pt, wt[:, kb, :],
                                rhs_src[:, kb, ic * 512:(ic + 1) * 512],
                                start=(kb == 0), stop=(kb == KBLK - 1),
                            )
                        nc.vector.tensor_copy(
                            dst[:, g, ic * 512:(ic + 1) * 512], pt)
                for jb in (g,):
                    for ic in range(2):
                        pt = ptile()
                        for kb in range(KBLK):
                            nc.tensor.matmul(
                                pt, xT[:, kb, jb * P:(jb + 1) * P],
                                wv_sb[:, kb, ic * 512:(ic + 1) * 512],
                                start=(kb == 0), stop=(kb == KBLK - 1),
                            )
                        nc.vector.tensor_copy(
                            V[:, jb, ic * 512:(ic + 1) * 512], pt)
        # ============ Stage 2 + 3 scope =================================
        with tc.tile_pool(name="s23", bufs=1) as s23:
            outT = s23.tile([P, G, S], fp32, name="outT")   # 32 KB/p

            # ---- Stage 2: attention per head pair ----
            with tc.tile_pool(name="expp", bufs=2) as expp, \
                 tc.tile_pool(name="sr", bufs=2) as srpool:
                for g in range(G):
                    if LVL < 1:
                        # proj-only: consume QT/KT/V cheaply into outT
                        nc.vector.tensor_copy(outT[:, g, :], QT[:, g, :])
                        nc.vector.tensor_tensor(
                            outT[:, g, 0:S], outT[:, g, 0:S],
                            KT[:, g, 0:S], add)
                        nc.vector.tensor_tensor(
                            outT[:, g, 0:E], outT[:, g, 0:E],
                            V[:, g, 0:E], add)
                        continue
                    eP = [None, None]
                    for half in range(2):
                        eP[half] = expp.tile([P, NBLK, S], fp16, tag="ept",
                                             name=f"eP{g}_{half}")
                        lo = D * half
                        for J in range(NBLK):
                            pa = qtile()
                            for ic in range(2):
                                nc.tensor.matmul(
                                    pa[:, ic * 512:(ic + 1) * 512],
                                    KT[lo:lo + D, g, J * P:(J + 1) * P],
                                    QT[lo:lo + D, g, ic * 512:(ic + 1) * 512],
                                    start=True, stop=True,
                                    skip_group_check=True,
                                )
                            nc.scalar.activation(
                                out=eP[half][:, J, :],
                                in_=pa, func=Exp, scale=SCALE,
                            )
                    if LVL < 2:
                        # consume exp output cheaply into outT
                        nc.vector.tensor_copy(outT[:, g, :], eP[0][:, 0, :])
                        nc.vector.tensor_tensor(
                            outT[:, g, :], outT[:, g, :], eP[1][:, 0, :], add)
                        continue
                    for ic in range(2):
                        ps = ptile()
                        pp = ptile()
                        pb = ptile() if LVL >= 3 else None
                        for half in range(2):
                            hh = 2 * g + half
                            lo = D * half
                            c_h = 15360 - 1024 * hh
                            for J in range(NBLK):
                                rhs = eP[half][:, J, ic * 512:(ic + 1) * 512]
                                if True:
                                    nc.tensor.matmul(
                                        ps[lo:lo + D, :], ones16, rhs,
                                        start=(J == 0), stop=(J == NBLK - 1),
                                        skip_group_check=True,
                                    )
                                if True:
                                    nc.tensor.matmul(
                                        pp[lo:lo + D, :],
                                        V[:, J, hh * D:(hh + 1) * D], rhs,
                                        start=(J == 0), stop=(J == NBLK - 1),
                                        skip_group_check=True,
                                    )
                                if LVL >= 3:
                                    t2v = bass.AP(
                                        tensor=T2.tensor,
                                        offset=T2.offset + c_h + 8192 * ic + P * J,
                                        ap=[T2.ap[0], [1024, 8], [16, 64]],
                                    )
                                    nc.tensor.matmul(
                                        pb[lo:lo + D, :],
                                        V[:, J, hh * D:(hh + 1) * D], t2v,
                                        start=(J == 0), stop=(J == NBLK - 1),
                                        skip_group_check=True,
                                    )
                        srec = srpool.tile([P, 512], fp32, tag="srt")
                        nc.vector.reciprocal(srec, ps)
                        dstc = outT[:, g, ic * 512:(ic + 1) * 512]
                        nc.vector.tensor_tensor(dstc, pp, srec, mult)
                        if LVL >= 3:
                            nc.vector.tensor_tensor(dstc, dstc, pb, add)

            # ---- Stage 3: transpose + LayerNorm + store ----
            with tc.tile_pool(name="ln", bufs=3) as ln:
                for T in range(NBLK):
                    tmp = ln.tile([P, E], fp32, tag="tmpT")
                    pls = []
                    for c in range(2):
                        pl = ptile()
                        pls.append(pl)
                        for gg in range(4):
                            g = 4 * c + gg
                            # gather block-T columns contiguously (stationary
                            # matmul operands need a single free dim)
                            src = bass.AP(
                                tensor=outT.tensor,
                                offset=outT.offset + g * S + (56 - 8 * T),
                                ap=[outT.ap[0], [64, 16], [1, 8]],
                            )
                            nc.gpsimd.tensor_copy(
                                tmp[:, g * P:(g + 1) * P], src)
                            nc.tensor.matmul(
                                pl[:, gg * P:(gg + 1) * P],
                                tmp[:, g * P:(g + 1) * P], ident,
                                is_transpose=True, skip_group_check=True,
                            )
                    stats = ln.tile([P, 2, 6], fp32, tag="stats")
                    mv = ln.tile([P, 2], fp32, tag="mv")
                    for c in range(2):
                        nc.vector.bn_stats(stats[:, c, :], pls[c])
                    nc.vector.bn_aggr(mv, stats)
                    rstd = ln.tile([P, 1], fp32, tag="rstd")
                    murs = ln.tile([P, 1], fp32, tag="murs")
                    nc.scalar.activation(out=rstd, in_=mv[:, 1:2],
                                         func=Sqrt, bias=epsT, scale=1.0)
                    nc.vector.reciprocal(rstd, rstd)
                    nc.vector.tensor_tensor(murs, mv[:, 0:1], rstd, mult)
                    of = ln.tile([P, E], fp32, tag="of")
                    for c in range(2):
                        nc.vector.tensor_scalar(
                            of[:, c * 512:(c + 1) * 512], pls[c], rstd, murs,
                            op0=mult, op1=sub)
                    if not trivial_ln:
                        nc.vector.tensor_tensor(of, of, gamT, mult)
                        nc.vector.tensor_tensor(of, of, betT, add)
                    nc.sync.dma_start(out[T * P:(T + 1) * P, :], of)

    nc.compile()
    return nc


def get_nc(trivial_ln: bool = True):
    if trivial_ln not in _BUILT:
        _BUILT[trivial_ln] = _build(trivial_ln)
    return _BUILT[trivial_ln]


def make_in_maps(inputs):
    x = np.asarray(inputs["x"])
    rel = np.asarray(inputs["rel_table"])
    gamma = np.asarray(inputs["gamma"])
    beta = np.asarray(inputs["beta"])
    trivial_ln = bool(np.all(gamma == 1.0) and np.all(beta == 0.0))

    x16 = x.astype(np.float16)
    xt16 = np.ascontiguousarray(x16.transpose(0, 2, 1))          # (B, E, S)
    xtp16 = np.ascontiguousarray(x16[:, SIGMA, :].transpose(0, 2, 1))
    wq16 = np.asarray(inputs["Wq"]).astype(np.float16)
    wk16 = np.asarray(inputs["Wk"]).astype(np.float16)
    wv16 = np.asarray(inputs["Wv"]).astype(np.float16)
    flat16 = np.ascontiguousarray(rel.reshape(-1).astype(np.float16))

    in_maps = []
    for b in range(x.shape[0]):
        m = {"xt16": xt16[b], "xtp16": xtp16[b],
             "wq16": wq16, "wk16": wk16, "wv16": wv16, "flat16": flat16}
        if not trivial_ln:
            m["gamma"] = gamma.reshape(1, E).astype(np.float32)
            m["beta"] = beta.reshape(1, E).astype(np.float32)
        in_maps.append(m)
    return in_maps, trivial_ln


def unpermute(raw):
    """raw: (..., S, E) with permuted rows -> natural row order."""
    unperm = (np.arange(0, S, P)[:, None] + ROWMAP[None, :]).reshape(-1)
    fixed = np.empty_like(raw)
    fixed[..., unperm, :] = raw
    return fixed


def kernel(**inputs) -> np.ndarray:
    from concourse import bass_utils

    in_maps, trivial_ln = make_in_maps(inputs)
    nc = get_nc(trivial_ln)
    res = bass_utils.run_bass_kernel_spmd(nc, in_maps,
                                          core_ids=list(range(len(in_maps))))
    outs = np.stack([r["out"] for r in res.results])
    return unpermute(outs).astype(np.float32)



# revision 2
# speedup vs baseline: 1.1614x; 1.1614x over previous
"""Trainium2 Bass kernel for nn_Attention_Rel_Scl (B=8,S=1024,E=1024,H=16).

Data-parallel over batch: one batch element per NeuronCore (8 cores).

v5: natural-layout attention + fine-grained software-pipelined emission.
  - exp(QK^T/sqrt(E)) is the *stationary* operand of PV / colsum / biasV
    matmuls, so those cost only (out free size) PE cycles and the result
    lands directly in natural [row, feature] orientation (no transposes,
    no gathers). V carries an interleaved 1.0 column per head so PV and
    the softmax denominator come from one moving stream.
  - Emission interleaves 2 QK+exp J-steps between every ~2us PE chunk
    (projection half-chains, PV half-blocks): the in-order engines then
    pace each other without head-of-line stalls; Act (the 133us exp
    budget) starts ~15us in and stays ~full.
  - QT/KT/VE psum->SBUF copies run on GpSimd (Pool) so the DVE's
    reciprocal (which waits on PV groups) never blocks them.
  - T2 (bias table, 63.7KB/part) is DMA-filled into the region freed by
    the projection inputs, overlapping the back half of stage 2.
  - bias[h,i,j] = flat[(16368-1024h) + 1024*(i%16) - 16*(i//16) + j]
    (flat = rel_table.reshape(-1), clip never fires); rows processed in
    order f -> SIGMA[f] = 16*(63-f%64) + f//64 make the bias block for
    (hh, F, J) the T2 view at offset 15360-1024*hh+2048*F+128*J with
    ap [[1,128],[1024,2],[16,64]], T2[p,w] = flat[p+w].
  - LayerNorm in natural layout; combine-add + normalize-apply on Pool,
    bn_stats/aggr/recip on DVE, Sqrt on Act. Contiguous output DMA; host
    un-permutes rows (SIGMA).
"""

import sys

if "/opt/trn_rl_repo" not in sys.path:
    sys.path.insert(0, "/opt/trn_rl_repo")

import numpy as np

B, S, E, H = 8, 1024, 1024, 16
D = E // H          # 64 head dim
P = 128             # partitions
G = H // 2          # 8 head pairs
NBLK = S // P       # 8 key/query blocks
KBLK = E // P       # 8 contraction blocks
EPS = 1e-3
SCALE = float(E) ** -0.5
FLAT = (2 * S - 1) * H   # 32752
T2W = 32625              # max free offset 32624 (+p<=127 -> 32751 = FLAT-1)
DE = D + 1               # 65: V column block plus ones column

_f = np.arange(S)
SIGMA = 16 * (63 - _f % 64) + _f // 64

_BUILT = {}


def _build(trivial_ln: bool):
    import concourse.bass as bass
    import concourse.tile as tile
    from concourse import bacc, mybir
    from contextlib import ExitStack

    fp16 = mybir.dt.float16
    fp32 = mybir.dt.float32
    Exp = mybir.ActivationFunctionType.Exp
    Sqrt = mybir.ActivationFunctionType.Sqrt
    mult = mybir.AluOpType.mult
    add = mybir.AluOpType.add
    sub = mybir.AluOpType.subtract

    nc = bacc.Bacc("TRN2", target_bir_lowering=False, debug=False,
                   num_devices=8)

    xt16 = nc.dram_tensor("xt16", [E, S], fp16, kind="ExternalInput").ap()
    xtp16 = nc.dram_tensor("xtp16", [E, S], fp16, kind="ExternalInput").ap()
    wq16 = nc.dram_tensor("wq16", [E, E], fp16, kind="ExternalInput").ap()
    wk16 = nc.dram_tensor("wk16", [E, E], fp16, kind="ExternalInput").ap()
    wv16 = nc.dram_tensor("wv16", [E, E], fp16, kind="ExternalInput").ap()
    flat16 = nc.dram_tensor("flat16", [FLAT], fp16, kind="ExternalInput").ap()
    if not trivial_ln:
        gam = nc.dram_tensor("gamma", [1, E], fp32, kind="ExternalInput").ap()
        bet = nc.dram_tensor("beta", [1, E], fp32, kind="ExternalInput").ap()
    out = nc.dram_tensor("out", [S, E], fp32, kind="ExternalOutput").ap()

    with tile.TileContext(nc) as tc, ExitStack() as ctx:
        persist = ctx.enter_context(tc.tile_pool(name="persist", bufs=1))
        QT = persist.tile([P, G, S], fp16, name="QT")
        KT = persist.tile([P, G, S], fp16, name="KT")
        VE = persist.tile([P, NBLK, H * DE], fp16, name="VE")
        natSB = persist.tile([P, NBLK, E], fp32, name="natSB")
        srecSB = persist.tile([P, G, 2, NBLK], fp32, name="srecSB")
        epsT = persist.tile([P, 1], fp32, name="epsT")

        nc.vector.memset(epsT, EPS)
        nc.vector.memset(
            bass.AP(tensor=VE.tensor, offset=VE.offset + D,
                    ap=[VE.ap[0], [H * DE, NBLK], [DE, H]]),
            1.0)

        if not trivial_ln:
            gamT = persist.tile([P, E], fp32, name="gamT")
            betT = persist.tile([P, E], fp32, name="betT")
            nc.sync.dma_start(
                out=gamT,
                in_=bass.AP(tensor=gam.tensor, offset=0, ap=[[0, P], [1, E]]),
            )
            nc.sync.dma_start(
                out=betT,
                in_=bass.AP(tensor=bet.tensor, offset=0, ap=[[0, P], [1, E]]),
            )

        expp = ctx.enter_context(tc.tile_pool(name="expp", bufs=2))
        psQK = ctx.enter_context(
            tc.tile_pool(name="psQK", bufs=2, space="PSUM"))
        pvp = ctx.enter_context(
            tc.tile_pool(name="pvp", bufs=1, space="PSUM"))

        eP = {}
        pools = {}

        # ---- emission helpers: each returns a list of closures ("chunks");
        # E-units (one QK J-step + exp) are interleaved between chunks.
        def proj_chunks(g, wdram, dst, rhs_src):
            wt = None

            def mk(ic):
                def go(state):
                    if ic == 0:
                        w = wpool.tile([P, KBLK, P], fp16, tag="wtile", name="wt")
                        nc.sync.dma_start(
                            out=w,
                            in_=wdram.rearrange(
                                "(kb kp) e -> kp kb e", kp=P)[
                                :, :, g * P:(g + 1) * P],
                        )
                        state["wt"] = w
                        state["pt"] = pools["psProj"].tile([P, E], fp32, tag="proj", name="pt")
                    w, pt = state["wt"], state["pt"]
                    for kb in range(KBLK):
                        nc.tensor.matmul(
                            pt[:, ic * 512:(ic + 1) * 512], w[:, kb, :],
                            rhs_src[:, kb, ic * 512:(ic + 1) * 512],
                            start=(kb == 0), stop=(kb == KBLK - 1),
                            skip_group_check=True,
                        )
                    if ic == 1:
                        nc.vector.tensor_copy(dst[:, g, :], pt)
                return go
            state = {}
            return [lambda s=state, f=mk(0): f(s),
                    lambda s=state, f=mk(1): f(s)]

        def vproj_chunks(jb):
            state = {}

            def mk(ic):
                def go(st):
                    if ic == 0:
                        st["pt"] = pools["psProj"].tile([P, E], fp32, tag="proj", name="pt")
                    pt = st["pt"]
                    for kb in range(KBLK):
                        nc.tensor.matmul(
                            pt[:, ic * 512:(ic + 1) * 512],
                            xT[:, kb, jb * P:(jb + 1) * P],
                            wv_sb[:, kb, ic * 512:(ic + 1) * 512],
                            start=(kb == 0), stop=(kb == KBLK - 1),
                            skip_group_check=True,
                        )
                    dstv = bass.AP(
                        tensor=VE.tensor,
                        offset=VE.offset + jb * (H * DE) + ic * 8 * DE,
                        ap=[VE.ap[0], [DE, 8], [1, D]],
                    )
                    nc.vector.tensor_copy(dstv,
                                          pt[:, ic * 512:(ic + 1) * 512])
                return go
            return [lambda f=mk(0): f(state), lambda f=mk(1): f(state)]

        def pv_chunks(g, half):
            u = 2 * g + half
            hh = u
            state = {}

            def mk(fh):
                def go(st):
                    if fh == 0:
                        st["pv"] = pvp.tile([P, NBLK, P], fp32, tag="pv", name="pv")
                    pv = st["pv"]
                    for F in range(4 * fh, 4 * fh + 4):
                        for J in range(NBLK):
                            nc.tensor.matmul(
                                pv[:, F, 0:DE],
                                eP[u][:, J, F * P:(F + 1) * P],
                                VE[:, J, hh * DE:(hh + 1) * DE],
                                start=(J == 0), stop=(J == NBLK - 1),
                                skip_group_check=True,
                            )
                    if fh == 1:
                        del eP[u]
                        srec = srecSB[:, g, half, :]
                        nc.vector.reciprocal(
                            srec,
                            bass.AP(tensor=pv.tensor, offset=pv.offset + D,
                                    ap=[pv.ap[0], [P, NBLK]]))
                        natv = bass.AP(
                            tensor=natSB.tensor,
                            offset=natSB.offset + hh * D,
                            ap=[natSB.ap[0], [E, NBLK], [1, D]],
                        )
                        pvv = bass.AP(tensor=pv.tensor, offset=pv.offset,
                                      ap=[pv.ap[0], [P, NBLK], [1, D]])
                        srecb = bass.AP(
                            tensor=srecSB.tensor,
                            offset=srecSB.offset + u * NBLK,
                            ap=[srecSB.ap[0], [1, NBLK], [0, D]],
                        )
                        nc.vector.tensor_tensor(natv, pvv, srecb, mult)
                return go
            return [lambda f=mk(0): f(state), lambda f=mk(1): f(state)]

        def e_units(g):
            units = []
            for half in range(2):
                for J in range(NBLK):
                    def go(half=half, J=J):
                        u = 2 * g + half
                        if J == 0:
                            eP[u] = expp.tile([P, NBLK, S], fp16, tag="ept", name="eP")
                        lo = D * half
                        pa = psQK.tile([P, E], fp32, tag="qk", name="pa")
                        for ic in range(2):
                            nc.tensor.matmul(
                                pa[:, ic * 512:(ic + 1) * 512],
                                KT[lo:lo + D, g, J * P:(J + 1) * P],
                                QT[lo:lo + D, g, ic * 512:(ic + 1) * 512],
                                start=True, stop=True,
                                skip_group_check=True,
                            )
                        nc.scalar.activation(
                            out=eP[u][:, J, :], in_=pa, func=Exp, scale=SCALE)
                    units.append(go)
            return units

        def emit_interleaved(chunks, units, gates=None):
            # spread E-units evenly between chunks; unit k may only be
            # emitted once gates[k] chunks are done (WAR: the eP slot it
            # allocates must have its reader PV already emitted).
            nc_, nu = len(chunks), len(units)
            if gates is None:
                gates = [0] * nu
            ui = 0
            for ci, ch in enumerate(chunks):
                ch()
                done = ci + 1
                want = done * nu // nc_
                while ui < want and ui < nu and gates[ui] <= done:
                    units[ui]()
                    ui += 1
            while ui < nu:
                units[ui]()
                ui += 1

        # ---- fused stage 1+2 ----
        with tc.tile_pool(name="psProj", bufs=1, space="PSUM") as psProj, \
             tc.tile_pool(name="s1fix", bufs=1) as s1fix, \
             tc.tile_pool(name="wpool", bufs=4) as wpool:
            pools["psProj"] = psProj
            xT = s1fix.tile([P, KBLK, S], fp16, name="xT")
            xTp = s1fix.tile([P, KBLK, S], fp16, name="xTp")
            wv_sb = s1fix.tile([P, KBLK, E], fp16, name="wv_sb")
            nc.sync.dma_start(
                out=xT, in_=xt16.rearrange("(kb kp) s -> kp kb s", kp=P))
            nc.sync.dma_start(
                out=xTp, in_=xtp16.rearrange("(kb kp) s -> kp kb s", kp=P))
            nc.sync.dma_start(
                out=wv_sb,
                in_=wv16.rearrange("(kb kp) e -> kp kb e", kp=P),
            )

            for i in range(G):
                chunks = []
                if 2 <= i:
                    chunks += pv_chunks(i - 2, 0) + pv_chunks(i - 2, 1)
                chunks += proj_chunks(i, wq16, QT, xTp)
                chunks += proj_chunks(i, wk16, KT, xT)
                if i == 0:
                    for jb in range(4):
                        chunks += vproj_chunks(jb)
                if i == 1:
                    for jb in range(4, 8):
                        chunks += vproj_chunks(jb)
                units = e_units(i - 1) if i >= 1 else []
                gates = ([2] * 8 + [4] * 8) if i >= 2 else [0] * len(units)
                emit_interleaved(chunks, units, gates)

        # ---- tail of stage 2 + stage 3 (T2 reuses the s1fix region) ----
        with tc.tile_pool(name="t2p", bufs=1) as t2p, \
             tc.tile_pool(name="ln", bufs=3) as ln:
            T2 = t2p.tile([P, T2W], fp16, name="T2")
            nc.sync.dma_start(
                out=T2,
                in_=bass.AP(tensor=flat16.tensor, offset=0,
                            ap=[[1, P], [1, T2W]]),
            )
            # iters 8, 9 of the pipeline
            chunks = pv_chunks(G - 2, 0) + pv_chunks(G - 2, 1)
            emit_interleaved(chunks, e_units(G - 1), [2] * 8 + [4] * 8)
            for c in pv_chunks(G - 1, 0) + pv_chunks(G - 1, 1):
                c()

            with tc.tile_pool(name="bps", bufs=1, space="PSUM") as bps:
                for F in range(NBLK):
                    # double-buffer the bias accumulator across the bps pool
                    # and the (now idle) pv pool's bank pair
                    if F % 2 == 0:
                        bias_ps = bps.tile([P, E], fp32, tag="bias",
                                           name="bias")
                    else:
                        bt = pvp.tile([P, NBLK, P], fp32, tag="pv",
                                      name="pv")
                        bias_ps = bass.AP(tensor=bt.tensor, offset=bt.offset,
                                          ap=[bt.ap[0], [1, E]])
                    for hh in range(H):
                        base = 15360 - 1024 * hh + 2048 * F
                        for J in range(NBLK):
                            t2st = bass.AP(
                                tensor=T2.tensor,
                                offset=T2.offset + base + P * J,
                                ap=[T2.ap[0], [1024, 2], [16, 64]],
                            )
                            nc.tensor.matmul(
                                bias_ps[:, hh * D:(hh + 1) * D], t2st,
                                VE[:, J, hh * DE:hh * DE + D],
                                start=(J == 0), stop=(J == NBLK - 1),
                                skip_group_check=True,
                            )
                    comb = ln.tile([P, E], fp32, tag="comb", name="comb")
                    nc.vector.tensor_tensor(comb, natSB[:, F, :], bias_ps,
                                            add)
                    stats = ln.tile([P, 2, 6], fp32, tag="stats", name="stats")
                    mv = ln.tile([P, 2], fp32, tag="mv", name="mv")
                    for c in range(2):
                        nc.vector.bn_stats(stats[:, c, :],
                                           comb[:, c * 512:(c + 1) * 512])
                    nc.vector.bn_aggr(mv, stats)
                    rstd = ln.tile([P, 1], fp32, tag="rstd", name="rstd")
                    murs = ln.tile([P, 1], fp32, tag="murs", name="murs")
                    nc.scalar.activation(out=rstd, in_=mv[:, 1:2],
                                         func=Sqrt, bias=epsT, scale=1.0)
                    nc.vector.reciprocal(rstd, rstd)
                    nc.vector.tensor_tensor(murs, mv[:, 0:1], rstd, mult)
                    of = ln.tile([P, E], fp32, tag="of", name="of")
                    nc.gpsimd.tensor_scalar(of, comb, rstd, murs,
                                            op0=mult, op1=sub)
                    if not trivial_ln:
                        nc.vector.tensor_tensor(of, of, gamT, mult)
                        nc.vector.tensor_tensor(of, of, betT, add)
                    nc.sync.dma_start(out[F * P:(F + 1) * P, :], of)

    nc.compile()
    return nc


def get_nc(trivial_ln: bool = True):
    if trivial_ln not in _BUILT:
        _BUILT[trivial_ln] = _build(trivial_ln)
    return _BUILT[trivial_ln]


def make_in_maps(inputs):
    x = np.asarray(inputs["x"])
    rel = np.asarray(inputs["rel_table"])
    gamma = np.asarray(inputs["gamma"])
    beta = np.asarray(inputs["beta"])
    trivial_ln = bool(np.all(gamma == 1.0) and np.all(beta == 0.0))

    x16 = x.astype(np.float16)
    xt16 = np.ascontiguousarray(x16.transpose(0, 2, 1))          # (B, E, S)
    xtp16 = np.ascontiguousarray(x16[:, SIGMA, :].transpose(0, 2, 1))
    wq16 = np.asarray(inputs["Wq"]).astype(np.float16)
    wk16 = np.asarray(inputs["Wk"]).astype(np.float16)
    wv16 = np.asarray(inputs["Wv"]).astype(np.float16)
    flat16 = np.ascontiguousarray(rel.reshape(-1).astype(np.float16))

    in_maps = []
    for b in range(x.shape[0]):
        m = {"xt16": xt16[b], "xtp16": xtp16[b],
             "wq16": wq16, "wk16": wk16, "wv16": wv16, "flat16": flat16}
        if not trivial_ln:
            m["gamma"] = gamma.reshape(1, E).astype(np.float32)
            m["beta"] = beta.reshape(1, E).astype(np.float32)
        in_maps.append(m)
    return in_maps, trivial_ln


def unpermute(raw):
    """raw: (..., S, E) rows in processing order -> natural order."""
    fixed = np.empty_like(raw)
    fixed[..., SIGMA, :] = raw
    return fixed


def kernel(**inputs) -> np.ndarray:
    from concourse import bass_utils

    in_maps, trivial_ln = make_in_maps(inputs)
    nc = get_nc(trivial_ln)
    res = bass_utils.run_bass_kernel_spmd(nc, in_maps,
                                          core_ids=list(range(len(in_maps))))
    outs = np.stack([r["out"] for r in res.results])
    return unpermute(outs).astype(np.float32)
